# revision 1
# baseline (speedup 1.0000x reference)
"""Trainium2 Bass kernel for nn_BatchedHomoModel_22179211116720 (GNN message passing).

Self-contained: hardcodes problem shapes; shards the seed-node batch (+ its
sampled blocks) across 8 NeuronCores (data parallel, no collectives); runs one
SPMD Bass/Tile program via concourse.bass_utils.run_bass_kernel_spmd.
"""

import hashlib
from contextlib import ExitStack

import numpy as np

import concourse.tile as tile
from concourse import bacc, bass, mybir
from concourse.bass_utils import run_bass_kernel_spmd




P = 128
NCORES = 8
D = 128
SUW = 8  # sub-chunks per wide elementwise chunk


def _rsqrt_deg(counts):
    return (1.0 / np.sqrt(np.maximum(counts, 1).astype(np.float64))).astype(np.float32)


def _group_positions(keys, num_keys):
    """For each element, its index within its key group (order of appearance)."""
    order = np.argsort(keys, kind="stable")
    sorted_keys = keys[order]
    counts = np.bincount(keys, minlength=num_keys)
    starts = np.concatenate([[0], np.cumsum(counts)[:-1]])
    pos_sorted = np.arange(keys.shape[0]) - starts[sorted_keys]
    pos = np.empty_like(pos_sorted)
    pos[order] = pos_sorted
    return pos


def build_plan(inputs):
    emb = np.asarray(inputs["emb"])
    nid_src1 = np.asarray(inputs["nid_src1"]).astype(np.int64)
    nid_src2 = np.asarray(inputs["nid_src2"]).astype(np.int64)
    nid_dst2 = np.asarray(inputs["nid_dst2"]).astype(np.int64)
    e1_src = np.asarray(inputs["e1_src"]).astype(np.int64)
    e1_dst = np.asarray(inputs["e1_dst"]).astype(np.int64)
    e2_src = np.asarray(inputs["e2_src"]).astype(np.int64)
    e2_dst = np.asarray(inputs["e2_dst"]).astype(np.int64)

    N1 = nid_src1.shape[0]
    N2 = nid_src2.shape[0]
    B = nid_dst2.shape[0]
    E1 = e1_src.shape[0]
    E2 = e2_src.shape[0]
    assert B % (P * NCORES) == 0
    T2 = B // (P * NCORES)  # seed tiles per core

    # ---- global degrees (same as reference's segment sums) ----
    cnt_out1 = np.bincount(e1_src, minlength=N1)
    cnt_in1 = np.bincount(e1_dst, minlength=N2)
    cnt_out2 = np.bincount(e2_src, minlength=N2)
    cnt_in2 = np.bincount(e2_dst, minlength=B)
    s1_edge = _rsqrt_deg(cnt_out1)[e1_src]  # per-e1-edge src scale
    s2_edge = _rsqrt_deg(cnt_out2)[e2_src]  # per-e2-edge src scale
    r1_node = _rsqrt_deg(cnt_in1)  # per-mid dst scale (global mid id)
    r2_node = _rsqrt_deg(cnt_in2)  # per-seed dst scale (orig position)

    # ---- seed permutation: sort by in-degree desc, deal into blocks ----
    seed_perm = np.argsort(-cnt_in2, kind="stable")  # rank -> orig seed pos
    rank_of_seed = np.empty(B, np.int64)
    rank_of_seed[seed_perm] = np.arange(B)
    # rank -> (g, p); g = c + 8*lt
    # K2 for local tile lt = max deg among blocks {8lt..8lt+7} = first seed of block 8lt
    K2 = []
    for lt in range(T2):
        g0 = 8 * lt
        K2.append(max(1, int(cnt_in2[seed_perm[P * g0]])))
    off2 = np.concatenate([[0], np.cumsum(K2)]).astype(np.int64)
    n2sub = int(off2[-1])
    n2sub_pad = ((n2sub + SUW - 1) // SUW) * SUW
    K2[-1] += n2sub_pad - n2sub  # extend last tile to SUW multiple
    off2 = np.concatenate([[0], np.cumsum(K2)]).astype(np.int64)
    n2sub = n2sub_pad

    # ---- e2 edge placement ----
    r_e = rank_of_seed[e2_dst]  # destination rank per e2 edge
    k_e = _group_positions(r_e, B)  # index within dst's edge list
    g_e = r_e // P
    p_e = r_e % P
    core_e = g_e % NCORES
    lt_e = g_e // NCORES
    su_e = off2[lt_e] + k_e
    assert (k_e < np.array(K2)[lt_e]).all()

    # ---- per-core mid sets ----
    mids_per_core = []
    for c in range(NCORES):
        mids = np.unique(e2_src[core_e == c])
        # sort by in-degree desc (stable for determinism)
        mids = mids[np.argsort(-cnt_in1[mids], kind="stable")]
        mids_per_core.append(mids)
    T1 = max((len(m) + P - 1) // P for m in mids_per_core)
    Mpad = T1 * P

    # K1 schedule: per core, per tile, max deg; then max across cores
    K1 = np.ones(T1, np.int64)
    for c in range(NCORES):
        degs = cnt_in1[mids_per_core[c]]
        for t in range((len(mids_per_core[c]) + P - 1) // P):
            K1[t] = max(K1[t], int(degs[t * P : (t + 1) * P].max()))
    off1 = np.concatenate([[0], np.cumsum(K1)]).astype(np.int64)
    n1sub = int(off1[-1])
    n1sub_pad = ((n1sub + SUW - 1) // SUW) * SUW
    K1[-1] += n1sub_pad - n1sub
    off1 = np.concatenate([[0], np.cumsum(K1)]).astype(np.int64)
    n1sub = n1sub_pad

    # ---- per-core metadata ----
    cores = []
    for c in range(NCORES):
        mids = mids_per_core[c]
        n_mid = len(mids)
        mid2local = np.full(N2, -1, np.int64)
        mid2local[mids] = np.arange(n_mid)

        # L1 edges for this core
        lm = mid2local[e1_dst]
        sel = lm >= 0
        idx = np.nonzero(sel)[0]
        lm_sel = lm[idx]
        k1 = _group_positions(lm_sel, n_mid)
        t1 = lm_sel // P
        p1 = lm_sel % P
        su1 = off1[t1] + k1
        assert (k1 < K1[t1]).all()

        s1meta = np.zeros((P, n1sub), np.float32)
        gidx1 = np.zeros((P, n1sub), np.int32)
        s1meta[p1, su1] = s1_edge[idx]
        gidx1[p1, su1] = nid_src1[e1_src[idx]]

        # L2 edges for this core
        sel2 = np.nonzero(core_e == c)[0]
        s2meta = np.zeros((P, n2sub), np.float32)
        gidxA = np.zeros((P, n2sub), np.int32)
        gidxB = np.zeros((P, n2sub), np.int32)
        s2meta[p_e[sel2], su_e[sel2]] = s2_edge[sel2]
        gidxA[p_e[sel2], su_e[sel2]] = nid_src2[e2_src[sel2]]
        gidxB[p_e[sel2], su_e[sel2]] = mid2local[e2_src[sel2]]

        # r1 per (slot, tile): slot p of tile t is local mid t*128+p
        r1row = np.ones(Mpad, np.float32)
        r1row[:n_mid] = r1_node[mids]
        r1meta = r1row.reshape(T1, P).T.copy()  # [P, T1]

        # seeds of this core: slot (lt, p) -> rank (c + 8*lt)*P + p
        lt_grid, p_grid = np.meshgrid(np.arange(T2), np.arange(P), indexing="ij")
        ranks = (c + NCORES * lt_grid) * P + p_grid  # [T2, P]
        orig = seed_perm[ranks]  # [T2, P] original seed positions
        h0meta = nid_dst2[orig].T.astype(np.int32).copy()  # [P, T2]
        r2meta = r2_node[orig].T.astype(np.float32).copy()  # [P, T2]

        cores.append(
            dict(
                s1meta=s1meta,
                gidx1=gidx1,
                s2meta=s2meta,
                gidxA=gidxA,
                gidxB=gidxB,
                r1meta=r1meta,
                h0meta=h0meta,
                r2meta=r2meta,
                orig_seeds=orig,  # [T2, P]
            )
        )

    return dict(
        cores=cores,
        K1=list(K1),
        K2=list(K2),
        off1=off1,
        off2=off2,
        T1=T1,
        T2=T2,
        Mpad=Mpad,
        n1sub=n1sub,
        n2sub=n2sub,
        seed_perm=seed_perm,
    )


def leaky(x):
    return np.maximum(x, 0.01 * x)


def simulate_plan(inputs, plan):
    """Numpy simulation of exactly what the device kernel will compute."""
    emb = np.asarray(inputs["emb"])
    W0 = np.asarray(inputs["W0"])
    b0 = np.asarray(inputs["b0"])
    W1 = np.asarray(inputs["W1"])
    b1 = np.asarray(inputs["b1"])
    T1, T2 = plan["T1"], plan["T2"]
    K1, K2 = plan["K1"], plan["K2"]
    off1, off2 = plan["off1"], plan["off2"]
    Mpad = plan["Mpad"]
    B = 4096

    out = np.zeros((B, D), np.float32)
    for c, md in enumerate(plan["cores"]):
        # ---- L1: aggregate into mids ----
        X = emb[md["gidx1"]]  # [P, n1sub, D]
        Y = leaky(X) * md["s1meta"][:, :, None]  # [P, n1sub, D]
        hmid = np.zeros((Mpad, D), np.float32)
        for t in range(T1):
            agg = Y[:, off1[t] : off1[t + 1], :].sum(axis=1)  # [P, D] per dst slot
            agg = agg * md["r1meta"][:, t, None]
            z = agg @ W0 + b0
            hmid[t * P : (t + 1) * P] = z  # pre-activation stored
        # ---- L2 ----
        XA = leaky(emb[md["gidxA"]]) * md["s2meta"][:, :, None]
        XB = leaky(hmid[md["gidxB"]]) * md["s2meta"][:, :, None]
        for t in range(T2):
            agg2 = (XA + XB)[:, off2[t] : off2[t + 1], :].sum(axis=1)  # [P, D]
            z2 = (agg2 @ W1) * md["r2meta"][:, t, None] + 2.0 * b1
            h0 = emb[md["h0meta"][:, t]]
            out[md["orig_seeds"][t]] = z2 + h0
    return out






F32 = mybir.dt.float32
I32 = mybir.dt.int32


def build_nc(plan, has_b0, has_b1, enable_asserts=False, variant="full"):
    T1, T2 = plan["T1"], plan["T2"]
    K1, K2 = [int(k) for k in plan["K1"]], [int(k) for k in plan["K2"]]
    off1 = [int(x) for x in plan["off1"]]
    off2 = [int(x) for x in plan["off2"]]
    Mpad = int(plan["Mpad"])
    n1sub, n2sub = int(plan["n1sub"]), int(plan["n2sub"])
    SUW = 8
    assert n1sub % SUW == 0 and n2sub % SUW == 0

    # su -> tile lookup
    tile_of_su1 = np.repeat(np.arange(T1), K1)
    tile_of_su2 = np.repeat(np.arange(T2), K2)

    nc = bacc.Bacc(
        "TRN2",
        target_bir_lowering=False,
        debug=False,
        enable_asserts=enable_asserts,
        num_devices=8,
    )

    # ---- DRAM tensors ----
    emb_d = nc.dram_tensor("emb", (500000, D), F32, kind="ExternalInput").ap()
    W0_d = nc.dram_tensor("W0", (D, D), F32, kind="ExternalInput").ap()
    W1_d = nc.dram_tensor("W1", (D, D), F32, kind="ExternalInput").ap()
    ident_d = nc.dram_tensor("ident", (P, P), F32, kind="ExternalInput").ap()
    gidx1_d = nc.dram_tensor("gidx1", (P, n1sub), I32, kind="ExternalInput").ap()
    s1_d = nc.dram_tensor("s1meta", (P, n1sub), F32, kind="ExternalInput").ap()
    gidxA_d = nc.dram_tensor("gidxA", (P, n2sub), I32, kind="ExternalInput").ap()
    gidxB_d = nc.dram_tensor("gidxB", (P, n2sub), I32, kind="ExternalInput").ap()
    s2_d = nc.dram_tensor("s2meta", (P, n2sub), F32, kind="ExternalInput").ap()
    r1_d = nc.dram_tensor("r1meta", (P, T1), F32, kind="ExternalInput").ap()
    h0_d = nc.dram_tensor("h0meta", (P, T2), I32, kind="ExternalInput").ap()
    r2_d = nc.dram_tensor("r2meta", (P, T2), F32, kind="ExternalInput").ap()
    if has_b0:
        b0bc_d = nc.dram_tensor("b0bc", (P, D), F32, kind="ExternalInput").ap()
    if has_b1:
        b1bc_d = nc.dram_tensor("b1bc", (P, D), F32, kind="ExternalInput").ap()
    out_d = nc.dram_tensor("out", (T2 * P, D), F32, kind="ExternalOutput").ap()
    hmid_d = nc.dram_tensor("hmid", (Mpad, D), F32, kind="Internal").ap()

    AX = bass.IndirectOffsetOnAxis

    with tile.TileContext(nc) as tc, ExitStack() as ctx:
        cpool = ctx.enter_context(tc.tile_pool(name="const", bufs=1))
        wpool = ctx.enter_context(tc.tile_pool(name="work", bufs=3))
        spool = ctx.enter_context(tc.tile_pool(name="small", bufs=3))
        ppool = ctx.enter_context(tc.tile_pool(name="psum", bufs=1, space="PSUM"))

        def load_const(ap_d, dtype):
            nm = "c_" + ap_d.name
            t = cpool.tile(list(ap_d.shape), dtype, name=nm, tag=nm)
            nc.sync.dma_start(out=t[:], in_=ap_d[:])
            return t

        ident = load_const(ident_d, F32)
        W0_s = load_const(W0_d, F32)
        W1_s = load_const(W1_d, F32)
        gidx1 = load_const(gidx1_d, I32)
        s1 = load_const(s1_d, F32)
        gidxA = load_const(gidxA_d, I32)
        gidxB = load_const(gidxB_d, I32)
        s2 = load_const(s2_d, F32)
        r1m = load_const(r1_d, F32)
        h0m = load_const(h0_d, I32)
        r2m = load_const(r2_d, F32)
        b0bc = load_const(b0bc_d, F32) if has_b0 else None
        b1bc = load_const(b1bc_d, F32) if has_b1 else None

        hmid_writes = []

        def leaky_chunk(table_ap, gidx_sb, su0, nsu, tagpfx):
            """Gather nsu sub-chunks from table and apply leaky_relu.
            Returns y tile [128, nsu*128]."""
            x = wpool.tile([P, SUW * P], F32, tag=tagpfx + "x")
            v = wpool.tile([P, SUW * P], F32, tag=tagpfx + "v")
            y = wpool.tile([P, SUW * P], F32, tag=tagpfx + "y")
            if variant != "nodma":
                for s in range(nsu):
                    nc.gpsimd.indirect_dma_start(
                        out=x[:, s * P : (s + 1) * P],
                        out_offset=None,
                        in_=table_ap[:],
                        in_offset=AX(ap=gidx_sb[:, su0 + s : su0 + s + 1], axis=0),
                    )
            else:
                nc.vector.tensor_copy(out=x[:, :P], in_=ident[:])
            w = nsu * P
            if variant == "dmaonly":
                return x
            nc.scalar.activation(
                out=v[:, :w], in_=x[:, :w],
                func=mybir.ActivationFunctionType.Copy, scale=0.01,
            )
            nc.vector.tensor_tensor(
                out=y[:, :w], in0=x[:, :w], in1=v[:, :w], op=mybir.AluOpType.max
            )
            return y

        def diag_chunk(s_sb, su0, nsu):
            dg = wpool.tile([P, SUW * P], F32, tag="diag")
            w = nsu * P
            nc.vector.tensor_tensor(
                out=dg[:, :w],
                in0=ident[:].unsqueeze(1).to_broadcast([P, nsu, P]),
                in1=s_sb[:, su0 : su0 + nsu].unsqueeze(2).to_broadcast([P, nsu, P]),
                op=mybir.AluOpType.mult,
            )
            return dg

        # ================= L1 =================
        skip_mm = variant in ("dmaonly", "nomm")
        agg_ps = {}
        for w in range(n1sub // SUW):
            su0 = w * SUW
            y = leaky_chunk(emb_d, gidx1, su0, SUW, "")
            if skip_mm:
                continue
            dg = diag_chunk(s1, su0, SUW)
            for s in range(SUW):
                su = su0 + s
                t = int(tile_of_su1[su])
                k = su - off1[t]
                if k == 0:
                    agg_ps[t] = ppool.tile([P, P], F32, tag="agg1", bufs=3, name=f"agg1_{t}")
                nc.tensor.matmul(
                    out=agg_ps[t][:],
                    lhsT=y[:, s * P : (s + 1) * P],
                    rhs=dg[:, s * P : (s + 1) * P],
                    start=(k == 0),
                    stop=(k == K1[t] - 1),
                )
                if k == K1[t] - 1:
                    # epilogue for mid tile t
                    aggs = spool.tile([P, P], F32, tag="aggs")
                    nc.vector.tensor_copy(out=aggs[:], in_=agg_ps[t][:])
                    zp = ppool.tile([P, P], F32, tag="wout", bufs=1)
                    nc.tensor.matmul(
                        out=zp[:], lhsT=aggs[:], rhs=W0_s[:], start=True, stop=True
                    )
                    zt = spool.tile([P, P], F32, tag="zt")
                    nc.scalar.activation(
                        out=zt[:], in_=zp[:],
                        func=mybir.ActivationFunctionType.Copy,
                        scale=r1m[:, t : t + 1],
                    )
                    if has_b0:
                        nc.vector.tensor_tensor(
                            out=zt[:], in0=zt[:], in1=b0bc[:], op=mybir.AluOpType.add
                        )
                    wi = nc.sync.dma_start(
                        out=hmid_d[t * P : (t + 1) * P, :], in_=zt[:]
                    )
                    hmid_writes.append(wi)
                    del agg_ps[t]

        # ================= L2 =================
        agg2_ps = {}
        # stream A: emb gathers (independent of hmid)
        for w in range(n2sub // SUW):
            su0 = w * SUW
            yA = leaky_chunk(emb_d, gidxA, su0, SUW, "")
            if skip_mm:
                continue
            dgA = diag_chunk(s2, su0, SUW)
            for s in range(SUW):
                su = su0 + s
                t = int(tile_of_su2[su])
                k = su - off2[t]
                if k == 0:
                    agg2_ps[t] = ppool.tile([P, P], F32, tag="agg2", bufs=4, name=f"agg2_{t}")
                nc.tensor.matmul(
                    out=agg2_ps[t][:],
                    lhsT=yA[:, s * P : (s + 1) * P],
                    rhs=dgA[:, s * P : (s + 1) * P],
                    start=(k == 0),
                    stop=False,
                )
        # stream B: hmid gathers (must wait for all hmid writes)
        for w in range(n2sub // SUW):
            su0 = w * SUW
            yB = leaky_chunk(emb_d if skip_mm else hmid_d, gidxB, su0, SUW, "b")
            if skip_mm:
                continue
            dgB = diag_chunk(s2, su0, SUW)
            for s in range(SUW):
                su = su0 + s
                t = int(tile_of_su2[su])
                k = su - off2[t]
                nc.tensor.matmul(
                    out=agg2_ps[t][:],
                    lhsT=yB[:, s * P : (s + 1) * P],
                    rhs=dgB[:, s * P : (s + 1) * P],
                    start=False,
                    stop=(k == K2[t] - 1),
                )
        # epilogues
        if skip_mm:
            dummy = spool.tile([P, P], F32, tag="ot", name="dummy")
            nc.vector.tensor_copy(out=dummy[:], in_=ident[:])
            nc.sync.dma_start(out=out_d[0:P, :], in_=dummy[:])
        for t in range(T2 if not skip_mm else 0):
            agg2s = spool.tile([P, P], F32, tag="agg2s")
            nc.vector.tensor_copy(out=agg2s[:], in_=agg2_ps[t][:])
            op_ = ppool.tile([P, P], F32, tag="wout", bufs=1)
            nc.tensor.matmul(
                out=op_[:], lhsT=agg2s[:], rhs=W1_s[:], start=True, stop=True
            )
            h0t = spool.tile([P, P], F32, tag="h0t")
            nc.gpsimd.indirect_dma_start(
                out=h0t[:],
                out_offset=None,
                in_=emb_d[:],
                in_offset=AX(ap=h0m[:, t : t + 1], axis=0),
            )
            ot = spool.tile([P, P], F32, tag="ot")
            nc.scalar.activation(
                out=ot[:], in_=op_[:],
                func=mybir.ActivationFunctionType.Copy,
                scale=r2m[:, t : t + 1],
            )
            nc.vector.tensor_tensor(
                out=ot[:], in0=ot[:], in1=h0t[:], op=mybir.AluOpType.add
            )
            if has_b1:
                nc.vector.tensor_tensor(
                    out=ot[:], in0=ot[:], in1=b1bc[:], op=mybir.AluOpType.add
                )
            nc.sync.dma_start(out=out_d[t * P : (t + 1) * P, :], in_=ot[:])

    nc.compile()
    return nc


def make_in_maps(inputs, plan, has_b0, has_b1):
    ident = np.eye(P, dtype=np.float32)
    W0 = np.asarray(inputs["W0"], np.float32)
    W1 = np.asarray(inputs["W1"], np.float32)
    emb = np.ascontiguousarray(np.asarray(inputs["emb"], np.float32))
    in_maps = []
    for md in plan["cores"]:
        m = dict(
            emb=emb,
            W0=W0,
            W1=W1,
            ident=ident,
            gidx1=md["gidx1"],
            s1meta=md["s1meta"],
            gidxA=md["gidxA"],
            gidxB=md["gidxB"],
            s2meta=md["s2meta"],
            r1meta=md["r1meta"],
            h0meta=md["h0meta"],
            r2meta=md["r2meta"],
        )
        if has_b0:
            m["b0bc"] = np.broadcast_to(
                np.asarray(inputs["b0"], np.float32), (P, D)
            ).copy()
        if has_b1:
            m["b1bc"] = np.broadcast_to(
                2.0 * np.asarray(inputs["b1"], np.float32), (P, D)
            ).copy()
        in_maps.append(m)
    return in_maps


def assemble_output(plan, core_outs):
    B = 4096
    out = np.zeros((B, D), np.float32)
    for c, md in enumerate(plan["cores"]):
        co = core_outs[c]  # [T2*128, 128]
        for t in range(plan["T2"]):
            out[md["orig_seeds"][t]] = co[t * P : (t + 1) * P]
    return out


# ----------------- runner -----------------
_CACHE = {}


def _plan_key(inputs):
    h = hashlib.sha1()
    for k in ("nid_src1", "nid_src2", "nid_dst2", "e1_src", "e1_dst", "e2_src", "e2_dst", "b0", "b1"):
        a = np.ascontiguousarray(np.asarray(inputs[k]))
        h.update(k.encode())
        h.update(str(a.shape).encode())
        h.update(a.tobytes())
    return h.hexdigest()


def _get_compiled(inputs):
    key = _plan_key(inputs)
    if key not in _CACHE:
        pl = build_plan(inputs)
        has_b0 = bool(np.any(np.asarray(inputs["b0"]) != 0))
        has_b1 = bool(np.any(np.asarray(inputs["b1"]) != 0))
        nc = build_nc(pl, has_b0, has_b1)
        _CACHE[key] = (pl, has_b0, has_b1, nc)
    return _CACHE[key]


def run_kernel(inputs, trace=False, tmpdir=None):
    pl, has_b0, has_b1, nc = _get_compiled(inputs)
    in_maps = make_in_maps(inputs, pl, has_b0, has_b1)
    res = run_bass_kernel_spmd(
        nc, in_maps, core_ids=list(range(NCORES)), trace=trace, tmpdir=tmpdir
    )
    core_outs = [res.results[c]["out"] for c in range(NCORES)]
    out = assemble_output(pl, core_outs)
    return out, res


def kernel(**inputs):
    out, _ = run_kernel(inputs, trace=False)
    return out



# revision 2
# speedup vs baseline: 1.0154x; 1.0154x over previous
"""Trainium2 Bass kernel v2 for nn_BatchedHomoModel_22179211116720 (GNN message passing).

Data-parallel across 8 NeuronCores (seed batch + blocks sharded by seed; no
collectives). Per-core compacted bf16 embedding tables (row sharding) so all
row gathers go through dma_gather (InstDMAGatherAnt, int16 indices) on 4 SWDGE
queues -- 8 Q7 cores generate descriptors in parallel instead of 2.

Aggregation: edge rows land dst-slot-aligned [p=dst slot, su column]; per-column
diagonal scale matrices feed PE matmuls that accumulate per-dst sums in PSUM
(bf16 operands, fp32 accumulate). Layer-2 uses a per-mid pre-activated table
hm2[m] = leaky(hmid[m]) + leaky(emb[nid_src2[m]]) so h1+h2 collapse into one
edge stream.
"""

import hashlib
from contextlib import ExitStack

import ml_dtypes
import numpy as np

import concourse.tile as tile
from concourse import bacc, bass, mybir
from concourse import library_config
from concourse.bass_utils import run_bass_kernel_spmd

P = 128
NCORES = 8
D = 128
CALLW1 = 8  # L1 gather call width (columns)
CALLW2 = 7  # L2 gather call width
NQ = 4  # SWDGE queues

F32 = mybir.dt.float32
BF16 = mybir.dt.bfloat16
I16 = mybir.dt.int16
BF16_NP = ml_dtypes.bfloat16


def _rsqrt_deg(counts):
    return (1.0 / np.sqrt(np.maximum(counts, 1).astype(np.float64))).astype(np.float32)


def _group_positions(keys, num_keys):
    order = np.argsort(keys, kind="stable")
    sorted_keys = keys[order]
    counts = np.bincount(keys, minlength=num_keys)
    starts = np.concatenate([[0], np.cumsum(counts)[:-1]])
    pos_sorted = np.arange(keys.shape[0]) - starts[sorted_keys]
    pos = np.empty_like(pos_sorted)
    pos[order] = pos_sorted
    return pos


def cdiv(a, b):
    return (a + b - 1) // b


def call_segments(lo, hi, w):
    """Column ranges [cu0, cu0+n) of width <= w covering [lo, hi)."""
    segs = []
    cu0 = lo
    while cu0 < hi:
        n = min(w, hi - cu0)
        segs.append((cu0, n))
        cu0 += n
    return segs


def wrap_idxs(flat):
    """Flat landing-order int array (len mult of 128) -> [128, len/16] int16
    (index j at [j%16, j//16], replicated across the 8 groups of 16)."""
    n = len(flat)
    assert n % 128 == 0
    a = np.asarray(flat, np.int16).reshape(n // 16, 16).T  # [16, n/16]
    return np.tile(a, (8, 1)).copy()  # [128, n/16]


def build_plan(inputs):
    nid_src1 = np.asarray(inputs["nid_src1"]).astype(np.int64)
    nid_src2 = np.asarray(inputs["nid_src2"]).astype(np.int64)
    nid_dst2 = np.asarray(inputs["nid_dst2"]).astype(np.int64)
    e1_src = np.asarray(inputs["e1_src"]).astype(np.int64)
    e1_dst = np.asarray(inputs["e1_dst"]).astype(np.int64)
    e2_src = np.asarray(inputs["e2_src"]).astype(np.int64)
    e2_dst = np.asarray(inputs["e2_dst"]).astype(np.int64)

    N1 = nid_src1.shape[0]
    N2 = nid_src2.shape[0]
    B = nid_dst2.shape[0]
    assert B % (P * NCORES) == 0
    T2 = B // (P * NCORES)

    cnt_out1 = np.bincount(e1_src, minlength=N1)
    cnt_in1 = np.bincount(e1_dst, minlength=N2)
    cnt_out2 = np.bincount(e2_src, minlength=N2)
    cnt_in2 = np.bincount(e2_dst, minlength=B)
    s1_edge = _rsqrt_deg(cnt_out1)[e1_src]
    s2_edge = _rsqrt_deg(cnt_out2)[e2_src]
    r1_node = _rsqrt_deg(cnt_in1)
    r2_node = _rsqrt_deg(cnt_in2)

    # seeds: sort by in-degree desc, deal into (core, lt, p)
    seed_perm = np.argsort(-cnt_in2, kind="stable")
    rank_of_seed = np.empty(B, np.int64)
    rank_of_seed[seed_perm] = np.arange(B)
    K2 = []
    for lt in range(T2):
        K2.append(max(1, int(cnt_in2[seed_perm[P * 8 * lt]])))
    n2sub = int(np.sum(K2))
    pad2 = (-n2sub) % CALLW2
    K2[-1] += pad2
    off2 = np.concatenate([[0], np.cumsum(K2)]).astype(np.int64)
    n2sub = int(off2[-1])

    # e2 edge placement
    r_e = rank_of_seed[e2_dst]
    k_e = _group_positions(r_e, B)
    g_e = r_e // P
    p_e = r_e % P
    core_e = g_e % NCORES
    lt_e = g_e // NCORES
    su_e = off2[lt_e] + k_e
    assert (k_e < np.array(K2)[lt_e]).all()

    # per-core mid sets (sorted by in-degree desc)
    mids_per_core = []
    for c in range(NCORES):
        mids = np.unique(e2_src[core_e == c])
        mids = mids[np.argsort(-cnt_in1[mids], kind="stable")]
        mids_per_core.append(mids)
    T1 = max((len(m) + P - 1) // P for m in mids_per_core)
    Mpad = T1 * P

    K1 = np.ones(T1, np.int64)
    for c in range(NCORES):
        degs = cnt_in1[mids_per_core[c]]
        for t in range((len(mids_per_core[c]) + P - 1) // P):
            K1[t] = max(K1[t], int(degs[t * P : (t + 1) * P].max()))
    n1sub = int(K1.sum())
    K1[-1] += (-n1sub) % CALLW1
    off1 = np.concatenate([[0], np.cumsum(K1)]).astype(np.int64)
    n1sub = int(off1[-1])

    # split tiles into two table halves (int16 reach)
    # choose TSPLIT so both halves' unique-row counts fit in 32768
    cores = []
    TSPLIT = None
    for c in range(NCORES):
        mids = mids_per_core[c]
        n_mid = len(mids)
        mid2local = np.full(N2, -1, np.int64)
        mid2local[mids] = np.arange(n_mid)

        lm = mid2local[e1_dst]
        sel = np.nonzero(lm >= 0)[0]
        lm_sel = lm[sel]
        k1 = _group_positions(lm_sel, n_mid)
        t1 = lm_sel // P
        p1 = lm_sel % P
        su1 = off1[t1] + k1
        assert (k1 < K1[t1]).all()

        # slot grids; fold the dst-side rsqrt(deg_in) scale into the edge
        # scale so no per-tile scale is needed on device
        rowid1 = np.full((P, n1sub), -1, np.int64)  # emb row per L1 slot
        s1meta = np.zeros((P, n1sub), np.float32)
        rowid1[p1, su1] = nid_src1[e1_src[sel]]
        s1meta[p1, su1] = s1_edge[sel] * r1_node[e1_dst[sel]]

        # L2 slots
        sel2 = np.nonzero(core_e == c)[0]
        j2 = np.zeros((P, n2sub), np.int64)  # local mid id per L2 slot
        s2meta = np.zeros((P, n2sub), np.float32)
        j2[p_e[sel2], su_e[sel2]] = mid2local[e2_src[sel2]]
        s2meta[p_e[sel2], su_e[sel2]] = s2_edge[sel2] * r2_node[e2_dst[sel2]]

        # mid emb rows per slot (t, p); pad -> -1
        midrow = np.full(Mpad, -1, np.int64)
        midrow[:n_mid] = nid_src2[mids]

        r1row = np.ones(Mpad, np.float32)
        r1row[:n_mid] = r1_node[mids]
        r1meta = r1row.reshape(T1, P).T.copy()

        lt_grid, p_grid = np.meshgrid(np.arange(T2), np.arange(P), indexing="ij")
        ranks = (c + NCORES * lt_grid) * P + p_grid
        orig = seed_perm[ranks]
        h0row = nid_dst2[orig]  # [T2, P]
        r2meta = r2_node[orig].T.astype(np.float32).copy()

        cores.append(
            dict(
                rowid1=rowid1, s1meta=s1meta,
                j2=j2, s2meta=s2meta,
                midrow=midrow, r1meta=r1meta,
                h0row=h0row, r2meta=r2meta, orig_seeds=orig,
            )
        )

    # pick TSPLIT: smallest t* such that both halves fit for every core
    def half_rows(md, t_lo, t_hi, with_h0):
        su_lo, su_hi = int(off1[t_lo]), int(off1[t_hi])
        rows = [md["rowid1"][:, su_lo:su_hi].ravel(), md["midrow"][t_lo * P : t_hi * P]]
        if with_h0:
            rows.append(md["h0row"].ravel())
        r = np.concatenate(rows)
        return np.unique(r[r >= 0])

    TSPLIT = T1 // 2
    for _ in range(T1):
        okA = all(len(half_rows(md, 0, TSPLIT, True)) <= 32768 for md in cores)
        okB = all(len(half_rows(md, TSPLIT, T1, False)) <= 32768 for md in cores)
        if okA and okB:
            break
        TSPLIT += -1 if not okA else 1
    assert all(len(half_rows(md, 0, TSPLIT, True)) <= 32768 for md in cores)
    assert all(len(half_rows(md, TSPLIT, T1, False)) <= 32768 for md in cores)

    # per-core compacted tables + int16 index grids
    nrowsA = nrowsB = 0
    for md in cores:
        rowsA = half_rows(md, 0, TSPLIT, True)
        rowsB = half_rows(md, TSPLIT, T1, False)
        md["rowsA"], md["rowsB"] = rowsA, rowsB
        nrowsA = max(nrowsA, len(rowsA))
        nrowsB = max(nrowsB, len(rowsB))

    suA = int(off1[TSPLIT])  # L1 columns in half A
    for md in cores:
        mapA = {r: i for i, r in enumerate(md["rowsA"])}
        mapB = {r: i for i, r in enumerate(md["rowsB"])}

        def enc(grid, m, su_lo, su_hi):
            g = grid[:, su_lo:su_hi]
            flat = g.ravel()
            tr = np.zeros(flat.shape, np.int64)
            valid = flat >= 0
            if valid.any():
                tr[valid] = np.array([m[r] for r in flat[valid]], np.int64)
            out = tr.reshape(g.shape).copy()
            out[g < 0] = -1
            return out

        idx1A = enc(md["rowid1"], mapA, 0, suA)  # [P, suA], -1 pads
        idx1B = enc(md["rowid1"], mapB, suA, n1sub)
        # fill pads: reuse previous real idx in the same column (or 0)
        for grid in (idx1A, idx1B):
            for su in range(grid.shape[1]):
                col = grid[:, su]
                bad = col < 0
                if bad.all():
                    col[:] = 0
                elif bad.any():
                    # fill from nearest valid above (cummax-style)
                    fill = col.copy()
                    last = col[~bad][0]
                    for p in range(P):
                        if fill[p] < 0:
                            fill[p] = last
                        else:
                            last = fill[p]
                    grid[:, su] = fill
        md["idx1A"], md["idx1B"] = idx1A, idx1B

        midA = np.array([mapA[r] if r >= 0 else 0 for r in md["midrow"][: TSPLIT * P]], np.int64)
        midB = np.array([mapB[r] if r >= 0 else 0 for r in md["midrow"][TSPLIT * P :]], np.int64)
        md["midA"], md["midB"] = midA, midB
        md["h0idx"] = np.array([mapA[r] for r in md["h0row"].ravel()], np.int64)  # lt-major

    return dict(
        cores=cores, K1=[int(k) for k in K1], K2=[int(k) for k in K2],
        off1=[int(x) for x in off1], off2=[int(x) for x in off2],
        T1=T1, T2=T2, TSPLIT=TSPLIT, suA=suA, Mpad=Mpad,
        n1sub=n1sub, n2sub=n2sub, nrowsA=nrowsA, nrowsB=nrowsB,
        seed_perm=seed_perm,
    )


def build_nc(plan, has_b0, has_b1):
    T1, T2 = plan["T1"], plan["T2"]
    K1, K2 = plan["K1"], plan["K2"]
    off1, off2 = plan["off1"], plan["off2"]
    TSPLIT, suA = plan["TSPLIT"], plan["suA"]
    n1sub, n2sub = plan["n1sub"], plan["n2sub"]
    nrowsA, nrowsB = plan["nrowsA"], plan["nrowsB"]
    Mpad = plan["Mpad"]

    tile_of_su1 = np.repeat(np.arange(T1), K1)
    tile_of_su2 = np.repeat(np.arange(T2), K2)

    nc = bacc.Bacc(
        "TRN2", target_bir_lowering=False, debug=False,
        num_devices=8, num_swdge_queues=NQ,
        dynamic_dma_scratch_size=32768,
    )

    tabA_d = nc.dram_tensor("tabA", (nrowsA, D), BF16, kind="ExternalInput").ap()
    tabB_d = nc.dram_tensor("tabB", (nrowsB, D), BF16, kind="ExternalInput").ap()
    W0_d = nc.dram_tensor("W0", (D, D), BF16, kind="ExternalInput").ap()
    W1_d = nc.dram_tensor("W1", (D, D), BF16, kind="ExternalInput").ap()
    ident_d = nc.dram_tensor("ident", (P, P), BF16, kind="ExternalInput").ap()
    s1_d = nc.dram_tensor("s1meta", (P, n1sub), BF16, kind="ExternalInput").ap()
    s2_d = nc.dram_tensor("s2meta", (P, n2sub), BF16, kind="ExternalInput").ap()
    r1_d = nc.dram_tensor("r1meta", (P, T1), F32, kind="ExternalInput").ap()
    r2_d = nc.dram_tensor("r2meta", (P, T2), F32, kind="ExternalInput").ap()
    ix1_d = nc.dram_tensor("ix1", (P, n1sub * 8), I16, kind="ExternalInput").ap()
    ixm_d = nc.dram_tensor("ixm", (P, Mpad // 16), I16, kind="ExternalInput").ap()
    ix2_d = nc.dram_tensor("ix2", (P, n2sub * 8), I16, kind="ExternalInput").ap()
    ixh_d = nc.dram_tensor("ixh", (P, T2 * 8), I16, kind="ExternalInput").ap()
    if has_b0:
        b0bc_d = nc.dram_tensor("b0bc", (P, D), F32, kind="ExternalInput").ap()
    if has_b1:
        b1bc_d = nc.dram_tensor("b1bc", (P, D), F32, kind="ExternalInput").ap()
    out_d = nc.dram_tensor("out", (T2 * P, D), F32, kind="ExternalOutput").ap()
    hm2_d = nc.dram_tensor("hm2", (Mpad, D), BF16, kind="Internal").ap()

    def nextq():
        # placeholder; real queue assigned post-scheduling to match the
        # Tile-assigned DMASW sem lane (sem lanes are locked to one queue)
        return 0

    with tile.TileContext(nc) as tc, ExitStack() as ctx:
        nc.gpsimd.load_library(library_config.mlp)
        cpool = ctx.enter_context(tc.tile_pool(name="const", bufs=1))
        xpool = ctx.enter_context(tc.tile_pool(name="xg", bufs=6))
        ypool = ctx.enter_context(tc.tile_pool(name="yg", bufs=4))
        dpool = ctx.enter_context(tc.tile_pool(name="dg", bufs=4))
        spool = ctx.enter_context(tc.tile_pool(name="small", bufs=3))
        ppool = ctx.enter_context(tc.tile_pool(name="psum", bufs=1, space="PSUM"))

        def load_const(ap_d, dtype):
            nm = "c_" + ap_d.name
            t = cpool.tile(list(ap_d.shape), dtype, name=nm, tag=nm)
            nc.sync.dma_start(out=t[:], in_=ap_d[:])
            return t

        def load_const_split(ap_d, dtype, pieces):
            nm = "c_" + ap_d.name
            t = cpool.tile(list(ap_d.shape), dtype, name=nm, tag=nm)
            n = ap_d.shape[1]
            step = cdiv(n, pieces)
            for o in range(0, n, step):
                e = min(o + step, n)
                nc.sync.dma_start(out=t[:, o:e], in_=ap_d[:, o:e])
            return t

        ident = load_const(ident_d, BF16)
        W0_s = load_const(W0_d, BF16)
        W1_s = load_const(W1_d, BF16)
        s1 = load_const(s1_d, BF16)
        s2 = load_const(s2_d, BF16)
        r1m = load_const(r1_d, F32)
        r2m = load_const(r2_d, F32)
        ix1 = load_const_split(ix1_d, I16, 4)
        ixm = load_const(ixm_d, I16)
        ix2 = load_const(ix2_d, I16)
        ixh = load_const(ixh_d, I16)
        b0bc = load_const(b0bc_d, F32) if has_b0 else None
        b1bc = load_const(b1bc_d, F32) if has_b1 else None

        def gather(out_ap_2d, ncols, table_ap, idx_slice):
            """One dma_gather: ncols*128 idxs -> out slice [128, ncols*128]."""
            nidx = ncols * P
            nc.gpsimd.dma_gather(
                out_ap_2d.rearrange("p (e q) -> p e q", q=P),
                table_ap[:],
                idx_slice,
                nidx,
                nidx,
                P,
                single_packet=False,
                queue_num=nextq(),
            )

        # ---- warmup: pay each Q7 pair's IRAM load off the critical path ----
        for _ in range(NQ):
            wt = spool.tile([P, P], BF16, tag="warm")
            gather(wt[:], 1, tabA_d, ixh[:, :8])

        # ---- h0 gather (early; table A) ----
        h0t = cpool.tile([P, T2 * P], BF16, tag="h0t")
        gather(h0t[:], T2, tabA_d, ixh[:, : T2 * 8])

        # ---- mid-emb gathers (early) -> leaky -> ym persistent ----
        ymraw = cpool.tile([P, Mpad], BF16, tag="ymraw")
        ym = cpool.tile([P, Mpad], BF16, tag="ym")
        MW = 12  # tiles per mid gather call
        for t0 in range(0, TSPLIT, MW):
            w = min(MW, TSPLIT - t0)
            gather(ymraw[:, t0 * P : (t0 + w) * P], w, tabA_d,
                   ixm[:, t0 * 8 : (t0 + w) * 8])
        for t0 in range(TSPLIT, T1, MW):
            w = min(MW, T1 - t0)
            gather(ymraw[:, t0 * P : (t0 + w) * P], w, tabB_d,
                   ixm[:, t0 * 8 : (t0 + w) * 8])
        nc.scalar.activation(
            out=ym[:], in_=ymraw[:],
            func=mybir.ActivationFunctionType.Lrelu, alpha=0.01,
        )

        # ---- L1 stream ----
        agg_ps = {}

        def l1_epilogue(t):
            aggs = spool.tile([P, P], BF16, tag="aggs")
            nc.vector.tensor_copy(out=aggs[:], in_=agg_ps[t][:])
            zp = ppool.tile([P, P], F32, tag="wout", bufs=1)
            nc.tensor.matmul(out=zp[:], lhsT=aggs[:], rhs=W0_s[:], start=True, stop=True)
            zt = spool.tile([P, P], BF16, tag="zt")
            if has_b0:
                ztf = spool.tile([P, P], F32, tag="ztf")
                nc.scalar.activation(
                    out=ztf[:], in_=zp[:],
                    func=mybir.ActivationFunctionType.Copy, scale=r1m[:, t : t + 1],
                )
                nc.vector.tensor_tensor(out=ztf[:], in0=ztf[:], in1=b0bc[:], op=mybir.AluOpType.add)
                nc.scalar.activation(
                    out=zt[:], in_=ztf[:],
                    func=mybir.ActivationFunctionType.Lrelu, alpha=0.01,
                )
            else:
                nc.scalar.activation(
                    out=zt[:], in_=zp[:],
                    func=mybir.ActivationFunctionType.Lrelu,
                    scale=r1m[:, t : t + 1], alpha=0.01,
                )
            hm = spool.tile([P, P], BF16, tag="hm")
            nc.vector.tensor_tensor(
                out=hm[:], in0=zt[:], in1=ym[:, t * P : (t + 1) * P], op=mybir.AluOpType.add
            )
            nc.sync.dma_start(out=hm2_d[t * P : (t + 1) * P, :], in_=hm[:])
            del agg_ps[t]

        for cu0, w in call_segments(0, suA, CALLW1) + call_segments(suA, n1sub, CALLW1):
            tab = tabA_d if cu0 < suA else tabB_d
            x = xpool.tile([P, w * P], BF16, tag="x1")
            gather(x[:], w, tab, ix1[:, cu0 * 8 : (cu0 + w) * 8])
            y = ypool.tile([P, w * P], BF16, tag="y1")
            nc.scalar.activation(
                out=y[:], in_=x[:],
                func=mybir.ActivationFunctionType.Lrelu, alpha=0.01,
            )
            dg = dpool.tile([P, w * P], BF16, tag="dg1")
            nc.vector.tensor_tensor(
                out=dg[:],
                in0=ident[:].unsqueeze(1).to_broadcast([P, w, P]),
                in1=s1[:, cu0 : cu0 + w].unsqueeze(2).to_broadcast([P, w, P]),
                op=mybir.AluOpType.mult,
            )
            for s in range(w):
                su = cu0 + s
                t = int(tile_of_su1[su])
                k = su - off1[t]
                if k == 0:
                    agg_ps[t] = ppool.tile([P, P], F32, tag="agg1", bufs=3, name=f"agg1_{t}")
                nc.tensor.matmul(
                    out=agg_ps[t][:],
                    lhsT=y[:, s * P : (s + 1) * P],
                    rhs=dg[:, s * P : (s + 1) * P],
                    start=(k == 0),
                    stop=(k == K1[t] - 1),
                )
                if k == K1[t] - 1:
                    l1_epilogue(t)

        # ---- L2 stream (gathers from hm2; waits on all hm2 writes) ----
        agg2_ps = {}
        for cu0, w in call_segments(0, n2sub, CALLW2):
            xh = xpool.tile([P, w * P], BF16, tag="x2")
            gather(xh[:], w, hm2_d, ix2[:, cu0 * 8 : (cu0 + w) * 8])
            dg2 = dpool.tile([P, w * P], BF16, tag="dg2")
            nc.vector.tensor_tensor(
                out=dg2[:],
                in0=ident[:].unsqueeze(1).to_broadcast([P, w, P]),
                in1=s2[:, cu0 : cu0 + w].unsqueeze(2).to_broadcast([P, w, P]),
                op=mybir.AluOpType.mult,
            )
            for s in range(w):
                su = cu0 + s
                t = int(tile_of_su2[su])
                k = su - off2[t]
                if k == 0:
                    agg2_ps[t] = ppool.tile([P, P], F32, tag="agg2", bufs=4, name=f"agg2_{t}")
                nc.tensor.matmul(
                    out=agg2_ps[t][:],
                    lhsT=xh[:, s * P : (s + 1) * P],
                    rhs=dg2[:, s * P : (s + 1) * P],
                    start=(k == 0),
                    stop=(k == K2[t] - 1),
                )

        for t in range(T2):
            a2 = spool.tile([P, P], BF16, tag="a2")
            nc.vector.tensor_copy(out=a2[:], in_=agg2_ps[t][:])
            op_ = ppool.tile([P, P], F32, tag="wout", bufs=1)
            nc.tensor.matmul(out=op_[:], lhsT=a2[:], rhs=W1_s[:], start=True, stop=True)
            ot = spool.tile([P, P], F32, tag="ot")
            nc.scalar.activation(
                out=ot[:], in_=op_[:],
                func=mybir.ActivationFunctionType.Copy, scale=r2m[:, t : t + 1],
            )
            h0f = spool.tile([P, P], F32, tag="h0f")
            nc.vector.tensor_copy(out=h0f[:], in_=h0t[:, t * P : (t + 1) * P])
            nc.vector.tensor_tensor(out=ot[:], in0=ot[:], in1=h0f[:], op=mybir.AluOpType.add)
            if has_b1:
                nc.vector.tensor_tensor(out=ot[:], in0=ot[:], in1=b1bc[:], op=mybir.AluOpType.add)
            nc.sync.dma_start(out=out_d[t * P : (t + 1) * P, :], in_=ot[:])

    # Assign each dma_gather's SWDGE queue from its Tile-assigned DMASW sem
    # lane (lane k -> queue k % NQ) so every sem lane sees exactly one queue.
    from concourse.tile_sem_assignment import PROC_NAME_TO_IDX

    dmasw_base = PROC_NAME_TO_IDX["DMASW0"]
    for inst in nc.inst_map.values():
        if isinstance(inst, mybir.InstDMAGatherAnt):
            proc = getattr(inst, "bass_scheduled_proc", None)
            if proc is not None and dmasw_base <= proc < dmasw_base + 8:
                inst.queue_num = (proc - dmasw_base) % NQ

    nc.compile()
    return nc


def make_in_maps(inputs, plan, has_b0, has_b1):
    emb = np.asarray(inputs["emb"], np.float32)
    W0 = np.asarray(inputs["W0"], np.float32).astype(BF16_NP)
    W1 = np.asarray(inputs["W1"], np.float32).astype(BF16_NP)
    ident = np.eye(P, dtype=np.float32).astype(BF16_NP)
    nrowsA, nrowsB = plan["nrowsA"], plan["nrowsB"]
    in_maps = []
    for md in plan["cores"]:
        tabA = np.zeros((nrowsA, D), BF16_NP)
        tabA[: len(md["rowsA"])] = emb[md["rowsA"]].astype(BF16_NP)
        tabB = np.zeros((nrowsB, D), BF16_NP)
        tabB[: len(md["rowsB"])] = emb[md["rowsB"]].astype(BF16_NP)

        # landing order = global column-major [su][p]; call segmentation is
        # column-contiguous so per-call slices of the wrapped array line up.
        grid = np.concatenate([md["idx1A"], md["idx1B"]], axis=1)  # [P, n1sub]
        ix1 = wrap_idxs(grid.T.ravel())
        ixm = wrap_idxs(np.concatenate([md["midA"], md["midB"]]))
        ix2 = wrap_idxs(md["j2"].T.ravel())
        ixh = wrap_idxs(md["h0idx"])

        m = dict(
            tabA=tabA, tabB=tabB, W0=W0, W1=W1, ident=ident,
            s1meta=md["s1meta"].astype(BF16_NP),
            s2meta=md["s2meta"].astype(BF16_NP),
            ix1=ix1, ixm=ixm, ix2=ix2, ixh=ixh,
        )
        if has_b0:
            m["b0bc"] = np.broadcast_to(np.asarray(inputs["b0"], np.float32), (P, D)).copy()
        if has_b1:
            m["b1bc"] = np.broadcast_to(2.0 * np.asarray(inputs["b1"], np.float32), (P, D)).copy()
        in_maps.append(m)
    return in_maps


def assemble_output(plan, core_outs):
    B = 4096
    out = np.zeros((B, D), np.float32)
    for c, md in enumerate(plan["cores"]):
        co = core_outs[c]
        for t in range(plan["T2"]):
            out[md["orig_seeds"][t]] = co[t * P : (t + 1) * P]
    return out


_CACHE = {}


def _plan_key(inputs):
    h = hashlib.sha1()
    for k in ("nid_src1", "nid_src2", "nid_dst2", "e1_src", "e1_dst", "e2_src", "e2_dst", "b0", "b1"):
        a = np.ascontiguousarray(np.asarray(inputs[k]))
        h.update(k.encode())
        h.update(str(a.shape).encode())
        h.update(a.tobytes())
    return h.hexdigest()


def _get_compiled(inputs):
    key = _plan_key(inputs)
    if key not in _CACHE:
        pl = build_plan(inputs)
        has_b0 = bool(np.any(np.asarray(inputs["b0"]) != 0))
        has_b1 = bool(np.any(np.asarray(inputs["b1"]) != 0))
        nc = build_nc(pl, has_b0, has_b1)
        _CACHE[key] = (pl, has_b0, has_b1, nc)
    return _CACHE[key]


def run_kernel(inputs, trace=False, tmpdir=None):
    pl, has_b0, has_b1, nc = _get_compiled(inputs)
    in_maps = make_in_maps(inputs, pl, has_b0, has_b1)
    res = run_bass_kernel_spmd(
        nc, in_maps, core_ids=list(range(NCORES)), trace=trace, tmpdir=tmpdir
    )
    core_outs = [res.results[c]["out"] for c in range(NCORES)]
    out = assemble_output(pl, core_outs)
    return out, res


def kernel(**inputs):
    out, _ = run_kernel(inputs, trace=False)
    return out


# revision 3
# speedup vs baseline: 1.0273x; 1.0117x over previous
"""Trainium2 Bass kernel v2 for nn_BatchedHomoModel_22179211116720 (GNN message passing).

Data-parallel across 8 NeuronCores (seed batch + blocks sharded by seed; no
collectives). Per-core compacted bf16 embedding tables (row sharding) so all
row gathers go through dma_gather (InstDMAGatherAnt, int16 indices) on 4 SWDGE
queues -- 8 Q7 cores generate descriptors in parallel instead of 2.

Aggregation: edge rows land dst-slot-aligned [p=dst slot, su column]; per-column
diagonal scale matrices feed PE matmuls that accumulate per-dst sums in PSUM
(bf16 operands, fp32 accumulate). Layer-2 uses a per-mid pre-activated table
hm2[m] = leaky(hmid[m]) + leaky(emb[nid_src2[m]]) so h1+h2 collapse into one
edge stream.
"""

import hashlib
from contextlib import ExitStack

import ml_dtypes
import numpy as np

import concourse.tile as tile
from concourse import bacc, bass, mybir
from concourse import library_config
from concourse.bass_utils import run_bass_kernel_spmd

P = 128
NCORES = 8
D = 128
CALLW1 = 8  # L1 gather call width (columns)
CALLW2 = 7  # L2 gather call width
NQ = 4  # SWDGE queues

F32 = mybir.dt.float32
BF16 = mybir.dt.bfloat16
I16 = mybir.dt.int16
BF16_NP = ml_dtypes.bfloat16


def _rsqrt_deg(counts):
    return (1.0 / np.sqrt(np.maximum(counts, 1).astype(np.float64))).astype(np.float32)


def _group_positions(keys, num_keys):
    order = np.argsort(keys, kind="stable")
    sorted_keys = keys[order]
    counts = np.bincount(keys, minlength=num_keys)
    starts = np.concatenate([[0], np.cumsum(counts)[:-1]])
    pos_sorted = np.arange(keys.shape[0]) - starts[sorted_keys]
    pos = np.empty_like(pos_sorted)
    pos[order] = pos_sorted
    return pos


def cdiv(a, b):
    return (a + b - 1) // b


def call_segments(lo, hi, w):
    """Column ranges [cu0, cu0+n) of width <= w covering [lo, hi)."""
    segs = []
    cu0 = lo
    while cu0 < hi:
        n = min(w, hi - cu0)
        segs.append((cu0, n))
        cu0 += n
    return segs


def wrap_idxs(flat):
    """Flat landing-order int array (len mult of 128) -> [128, len/16] int16
    (index j at [j%16, j//16], replicated across the 8 groups of 16)."""
    n = len(flat)
    assert n % 128 == 0
    a = np.asarray(flat, np.int16).reshape(n // 16, 16).T  # [16, n/16]
    return np.tile(a, (8, 1)).copy()  # [128, n/16]


def build_plan(inputs):
    nid_src1 = np.asarray(inputs["nid_src1"]).astype(np.int64)
    nid_src2 = np.asarray(inputs["nid_src2"]).astype(np.int64)
    nid_dst2 = np.asarray(inputs["nid_dst2"]).astype(np.int64)
    e1_src = np.asarray(inputs["e1_src"]).astype(np.int64)
    e1_dst = np.asarray(inputs["e1_dst"]).astype(np.int64)
    e2_src = np.asarray(inputs["e2_src"]).astype(np.int64)
    e2_dst = np.asarray(inputs["e2_dst"]).astype(np.int64)

    N1 = nid_src1.shape[0]
    N2 = nid_src2.shape[0]
    B = nid_dst2.shape[0]
    assert B % (P * NCORES) == 0
    T2 = B // (P * NCORES)

    cnt_out1 = np.bincount(e1_src, minlength=N1)
    cnt_in1 = np.bincount(e1_dst, minlength=N2)
    cnt_out2 = np.bincount(e2_src, minlength=N2)
    cnt_in2 = np.bincount(e2_dst, minlength=B)
    s1_edge = _rsqrt_deg(cnt_out1)[e1_src]
    s2_edge = _rsqrt_deg(cnt_out2)[e2_src]
    r1_node = _rsqrt_deg(cnt_in1)
    r2_node = _rsqrt_deg(cnt_in2)

    # seeds: sort by in-degree desc, deal into (core, lt, p)
    seed_perm = np.argsort(-cnt_in2, kind="stable")
    rank_of_seed = np.empty(B, np.int64)
    rank_of_seed[seed_perm] = np.arange(B)
    K2 = []
    for lt in range(T2):
        K2.append(max(1, int(cnt_in2[seed_perm[P * 8 * lt]])))
    n2sub = int(np.sum(K2))
    pad2 = (-n2sub) % CALLW2
    K2[-1] += pad2
    off2 = np.concatenate([[0], np.cumsum(K2)]).astype(np.int64)
    n2sub = int(off2[-1])

    # e2 edge placement; snake-deal seed groups to cores (0..7,7..0,...) so
    # per-core unique-mid counts balance (core 0 otherwise gets the highest-
    # degree group of every block and alone inflates T1/K1)
    r_e = rank_of_seed[e2_dst]
    k_e = _group_positions(r_e, B)
    g_e = r_e // P
    p_e = r_e % P
    _blk = g_e // NCORES
    _pos = g_e % NCORES
    core_e = np.where(_blk % 2 == 0, _pos, NCORES - 1 - _pos)
    lt_e = _blk
    su_e = off2[lt_e] + k_e
    assert (k_e < np.array(K2)[lt_e]).all()

    # per-core mid sets (sorted by in-degree desc)
    mids_per_core = []
    for c in range(NCORES):
        mids = np.unique(e2_src[core_e == c])
        mids = mids[np.argsort(-cnt_in1[mids], kind="stable")]
        mids_per_core.append(mids)
    T1 = max((len(m) + P - 1) // P for m in mids_per_core)
    Mpad = T1 * P

    K1 = np.ones(T1, np.int64)
    for c in range(NCORES):
        degs = cnt_in1[mids_per_core[c]]
        for t in range((len(mids_per_core[c]) + P - 1) // P):
            K1[t] = max(K1[t], int(degs[t * P : (t + 1) * P].max()))
    n1sub = int(K1.sum())
    K1[-1] += (-n1sub) % CALLW1
    off1 = np.concatenate([[0], np.cumsum(K1)]).astype(np.int64)
    n1sub = int(off1[-1])

    # split tiles into two table halves (int16 reach)
    # choose TSPLIT so both halves' unique-row counts fit in 32768
    cores = []
    TSPLIT = None
    for c in range(NCORES):
        mids = mids_per_core[c]
        n_mid = len(mids)
        mid2local = np.full(N2, -1, np.int64)
        mid2local[mids] = np.arange(n_mid)

        lm = mid2local[e1_dst]
        sel = np.nonzero(lm >= 0)[0]
        lm_sel = lm[sel]
        k1 = _group_positions(lm_sel, n_mid)
        t1 = lm_sel // P
        p1 = lm_sel % P
        su1 = off1[t1] + k1
        assert (k1 < K1[t1]).all()

        # slot grids; fold the dst-side rsqrt(deg_in) scale into the edge
        # scale so no per-tile scale is needed on device
        rowid1 = np.full((P, n1sub), -1, np.int64)  # emb row per L1 slot
        s1meta = np.zeros((P, n1sub), np.float32)
        rowid1[p1, su1] = nid_src1[e1_src[sel]]
        s1meta[p1, su1] = s1_edge[sel] * r1_node[e1_dst[sel]]

        # L2 slots
        sel2 = np.nonzero(core_e == c)[0]
        j2 = np.zeros((P, n2sub), np.int64)  # local mid id per L2 slot
        s2meta = np.zeros((P, n2sub), np.float32)
        j2[p_e[sel2], su_e[sel2]] = mid2local[e2_src[sel2]]
        s2meta[p_e[sel2], su_e[sel2]] = s2_edge[sel2] * r2_node[e2_dst[sel2]]

        # mid emb rows per slot (t, p); pad -> -1
        midrow = np.full(Mpad, -1, np.int64)
        midrow[:n_mid] = nid_src2[mids]

        r1row = np.ones(Mpad, np.float32)
        r1row[:n_mid] = r1_node[mids]
        r1meta = r1row.reshape(T1, P).T.copy()

        lt_grid, p_grid = np.meshgrid(np.arange(T2), np.arange(P), indexing="ij")
        pos_grid = np.where(lt_grid % 2 == 0, c, NCORES - 1 - c)
        ranks = (pos_grid + NCORES * lt_grid) * P + p_grid
        orig = seed_perm[ranks]
        h0row = nid_dst2[orig]  # [T2, P]
        r2meta = r2_node[orig].T.astype(np.float32).copy()

        cores.append(
            dict(
                rowid1=rowid1, s1meta=s1meta,
                j2=j2, s2meta=s2meta,
                midrow=midrow, r1meta=r1meta,
                h0row=h0row, r2meta=r2meta, orig_seeds=orig,
            )
        )

    # pick TSPLIT: smallest t* such that both halves fit for every core
    def half_rows(md, t_lo, t_hi, with_h0):
        su_lo, su_hi = int(off1[t_lo]), int(off1[t_hi])
        rows = [md["rowid1"][:, su_lo:su_hi].ravel(), md["midrow"][t_lo * P : t_hi * P]]
        if with_h0:
            rows.append(md["h0row"].ravel())
        r = np.concatenate(rows)
        return np.unique(r[r >= 0])

    TSPLIT = T1 // 2
    for _ in range(T1):
        okA = all(len(half_rows(md, 0, TSPLIT, True)) <= 32768 for md in cores)
        okB = all(len(half_rows(md, TSPLIT, T1, False)) <= 32768 for md in cores)
        if okA and okB:
            break
        TSPLIT += -1 if not okA else 1
    assert all(len(half_rows(md, 0, TSPLIT, True)) <= 32768 for md in cores)
    assert all(len(half_rows(md, TSPLIT, T1, False)) <= 32768 for md in cores)

    # per-core compacted tables + int16 index grids
    nrowsA = nrowsB = 0
    for md in cores:
        rowsA = half_rows(md, 0, TSPLIT, True)
        rowsB = half_rows(md, TSPLIT, T1, False)
        md["rowsA"], md["rowsB"] = rowsA, rowsB
        nrowsA = max(nrowsA, len(rowsA))
        nrowsB = max(nrowsB, len(rowsB))

    suA = int(off1[TSPLIT])  # L1 columns in half A
    for md in cores:
        mapA = {r: i for i, r in enumerate(md["rowsA"])}
        mapB = {r: i for i, r in enumerate(md["rowsB"])}

        def enc(grid, m, su_lo, su_hi):
            g = grid[:, su_lo:su_hi]
            flat = g.ravel()
            tr = np.zeros(flat.shape, np.int64)
            valid = flat >= 0
            if valid.any():
                tr[valid] = np.array([m[r] for r in flat[valid]], np.int64)
            out = tr.reshape(g.shape).copy()
            out[g < 0] = -1
            return out

        idx1A = enc(md["rowid1"], mapA, 0, suA)  # [P, suA], -1 pads
        idx1B = enc(md["rowid1"], mapB, suA, n1sub)
        # fill pads: reuse previous real idx in the same column (or 0)
        for grid in (idx1A, idx1B):
            for su in range(grid.shape[1]):
                col = grid[:, su]
                bad = col < 0
                if bad.all():
                    col[:] = 0
                elif bad.any():
                    # fill from nearest valid above (cummax-style)
                    fill = col.copy()
                    last = col[~bad][0]
                    for p in range(P):
                        if fill[p] < 0:
                            fill[p] = last
                        else:
                            last = fill[p]
                    grid[:, su] = fill
        md["idx1A"], md["idx1B"] = idx1A, idx1B

        midA = np.array([mapA[r] if r >= 0 else 0 for r in md["midrow"][: TSPLIT * P]], np.int64)
        midB = np.array([mapB[r] if r >= 0 else 0 for r in md["midrow"][TSPLIT * P :]], np.int64)
        md["midA"], md["midB"] = midA, midB
        md["h0idx"] = np.array([mapA[r] for r in md["h0row"].ravel()], np.int64)  # lt-major

    return dict(
        cores=cores, K1=[int(k) for k in K1], K2=[int(k) for k in K2],
        off1=[int(x) for x in off1], off2=[int(x) for x in off2],
        T1=T1, T2=T2, TSPLIT=TSPLIT, suA=suA, Mpad=Mpad,
        n1sub=n1sub, n2sub=n2sub, nrowsA=nrowsA, nrowsB=nrowsB,
        seed_perm=seed_perm,
    )


def build_nc(plan, has_b0, has_b1):
    T1, T2 = plan["T1"], plan["T2"]
    K1, K2 = plan["K1"], plan["K2"]
    off1, off2 = plan["off1"], plan["off2"]
    TSPLIT, suA = plan["TSPLIT"], plan["suA"]
    n1sub, n2sub = plan["n1sub"], plan["n2sub"]
    nrowsA, nrowsB = plan["nrowsA"], plan["nrowsB"]
    Mpad = plan["Mpad"]

    tile_of_su1 = np.repeat(np.arange(T1), K1)
    tile_of_su2 = np.repeat(np.arange(T2), K2)

    nc = bacc.Bacc(
        "TRN2", target_bir_lowering=False, debug=False,
        num_devices=8, num_swdge_queues=NQ,
        dynamic_dma_scratch_size=32768,
    )

    tabA_d = nc.dram_tensor("tabA", (nrowsA, D), BF16, kind="ExternalInput").ap()
    tabB_d = nc.dram_tensor("tabB", (nrowsB, D), BF16, kind="ExternalInput").ap()
    W0_d = nc.dram_tensor("W0", (D, D), BF16, kind="ExternalInput").ap()
    W1_d = nc.dram_tensor("W1", (D, D), BF16, kind="ExternalInput").ap()
    ident_d = nc.dram_tensor("ident", (P, P), BF16, kind="ExternalInput").ap()
    s1_d = nc.dram_tensor("s1meta", (P, n1sub), BF16, kind="ExternalInput").ap()
    s2_d = nc.dram_tensor("s2meta", (P, n2sub), BF16, kind="ExternalInput").ap()
    r1_d = nc.dram_tensor("r1meta", (P, T1), F32, kind="ExternalInput").ap()
    r2_d = nc.dram_tensor("r2meta", (P, T2), F32, kind="ExternalInput").ap()
    ix1_d = nc.dram_tensor("ix1", (P, n1sub * 8), I16, kind="ExternalInput").ap()
    ixm_d = nc.dram_tensor("ixm", (P, Mpad // 16), I16, kind="ExternalInput").ap()
    ix2_d = nc.dram_tensor("ix2", (P, n2sub * 8), I16, kind="ExternalInput").ap()
    ixh_d = nc.dram_tensor("ixh", (P, T2 * 8), I16, kind="ExternalInput").ap()
    if has_b0:
        b0bc_d = nc.dram_tensor("b0bc", (P, D), F32, kind="ExternalInput").ap()
    if has_b1:
        b1bc_d = nc.dram_tensor("b1bc", (P, D), F32, kind="ExternalInput").ap()
    out_d = nc.dram_tensor("out", (T2 * P, D), F32, kind="ExternalOutput").ap()
    hm2_d = nc.dram_tensor("hm2", (Mpad, D), BF16, kind="Internal").ap()

    def nextq():
        # placeholder; real queue assigned post-scheduling to match the
        # Tile-assigned DMASW sem lane (sem lanes are locked to one queue)
        return 0

    with tile.TileContext(nc) as tc, ExitStack() as ctx:
        nc.gpsimd.load_library(library_config.mlp)
        cpool = ctx.enter_context(tc.tile_pool(name="const", bufs=1))
        xpool = ctx.enter_context(tc.tile_pool(name="xg", bufs=6))
        ypool = ctx.enter_context(tc.tile_pool(name="yg", bufs=4))
        dpool = ctx.enter_context(tc.tile_pool(name="dg", bufs=4))
        spool = ctx.enter_context(tc.tile_pool(name="small", bufs=3))
        ppool = ctx.enter_context(tc.tile_pool(name="psum", bufs=1, space="PSUM"))

        def load_const(ap_d, dtype):
            nm = "c_" + ap_d.name
            t = cpool.tile(list(ap_d.shape), dtype, name=nm, tag=nm)
            nc.sync.dma_start(out=t[:], in_=ap_d[:])
            return t

        def load_const_split(ap_d, dtype, pieces):
            nm = "c_" + ap_d.name
            t = cpool.tile(list(ap_d.shape), dtype, name=nm, tag=nm)
            n = ap_d.shape[1]
            step = cdiv(n, pieces)
            for o in range(0, n, step):
                e = min(o + step, n)
                nc.sync.dma_start(out=t[:, o:e], in_=ap_d[:, o:e])
            return t

        ident = load_const(ident_d, BF16)
        W0_s = load_const(W0_d, BF16)
        W1_s = load_const(W1_d, BF16)
        s1 = load_const(s1_d, BF16)
        s2 = load_const(s2_d, BF16)
        r1m = load_const(r1_d, F32)
        r2m = load_const(r2_d, F32)
        ix1 = load_const_split(ix1_d, I16, 4)
        ixm = load_const(ixm_d, I16)
        ix2 = load_const(ix2_d, I16)
        ixh = load_const(ixh_d, I16)
        b0bc = load_const(b0bc_d, F32) if has_b0 else None
        b1bc = load_const(b1bc_d, F32) if has_b1 else None

        def gather(out_ap_2d, ncols, table_ap, idx_slice):
            """One dma_gather: ncols*128 idxs -> out slice [128, ncols*128]."""
            nidx = ncols * P
            nc.gpsimd.dma_gather(
                out_ap_2d.rearrange("p (e q) -> p e q", q=P),
                table_ap[:],
                idx_slice,
                nidx,
                nidx,
                P,
                single_packet=False,
                queue_num=nextq(),
            )

        # ---- warmup: pay each Q7 pair's IRAM load off the critical path ----
        for _ in range(NQ):
            wt = spool.tile([P, P], BF16, tag="warm")
            gather(wt[:], 1, tabA_d, ixh[:, :8])

        # ---- h0 gather (early; table A) ----
        h0t = cpool.tile([P, T2 * P], BF16, tag="h0t")
        gather(h0t[:], T2, tabA_d, ixh[:, : T2 * 8])

        # ---- mid-emb gathers (early) -> leaky -> ym persistent ----
        ymraw = cpool.tile([P, Mpad], BF16, tag="ymraw")
        ym = cpool.tile([P, Mpad], BF16, tag="ym")
        MW = 12  # tiles per mid gather call
        for t0 in range(0, TSPLIT, MW):
            w = min(MW, TSPLIT - t0)
            gather(ymraw[:, t0 * P : (t0 + w) * P], w, tabA_d,
                   ixm[:, t0 * 8 : (t0 + w) * 8])
        for t0 in range(TSPLIT, T1, MW):
            w = min(MW, T1 - t0)
            gather(ymraw[:, t0 * P : (t0 + w) * P], w, tabB_d,
                   ixm[:, t0 * 8 : (t0 + w) * 8])
        nc.scalar.activation(
            out=ym[:], in_=ymraw[:],
            func=mybir.ActivationFunctionType.Lrelu, alpha=0.01,
        )

        # ---- L1 stream ----
        agg_ps = {}

        def l1_epilogue(t):
            aggs = spool.tile([P, P], BF16, tag="aggs")
            nc.vector.tensor_copy(out=aggs[:], in_=agg_ps[t][:])
            zp = ppool.tile([P, P], F32, tag="wout", bufs=1)
            nc.tensor.matmul(out=zp[:], lhsT=aggs[:], rhs=W0_s[:], start=True, stop=True)
            zt = spool.tile([P, P], BF16, tag="zt")
            if has_b0:
                ztf = spool.tile([P, P], F32, tag="ztf")
                nc.scalar.activation(
                    out=ztf[:], in_=zp[:],
                    func=mybir.ActivationFunctionType.Copy, scale=r1m[:, t : t + 1],
                )
                nc.vector.tensor_tensor(out=ztf[:], in0=ztf[:], in1=b0bc[:], op=mybir.AluOpType.add)
                nc.scalar.activation(
                    out=zt[:], in_=ztf[:],
                    func=mybir.ActivationFunctionType.Lrelu, alpha=0.01,
                )
            else:
                nc.scalar.activation(
                    out=zt[:], in_=zp[:],
                    func=mybir.ActivationFunctionType.Lrelu,
                    scale=r1m[:, t : t + 1], alpha=0.01,
                )
            hm = spool.tile([P, P], BF16, tag="hm")
            nc.vector.tensor_tensor(
                out=hm[:], in0=zt[:], in1=ym[:, t * P : (t + 1) * P], op=mybir.AluOpType.add
            )
            nc.sync.dma_start(out=hm2_d[t * P : (t + 1) * P, :], in_=hm[:])
            del agg_ps[t]

        for cu0, w in call_segments(0, suA, CALLW1) + call_segments(suA, n1sub, CALLW1):
            tab = tabA_d if cu0 < suA else tabB_d
            x = xpool.tile([P, w * P], BF16, tag="x1")
            gather(x[:], w, tab, ix1[:, cu0 * 8 : (cu0 + w) * 8])
            y = ypool.tile([P, w * P], BF16, tag="y1")
            nc.scalar.activation(
                out=y[:], in_=x[:],
                func=mybir.ActivationFunctionType.Lrelu, alpha=0.01,
            )
            dg = dpool.tile([P, w * P], BF16, tag="dg1")
            nc.vector.tensor_tensor(
                out=dg[:],
                in0=ident[:].unsqueeze(1).to_broadcast([P, w, P]),
                in1=s1[:, cu0 : cu0 + w].unsqueeze(2).to_broadcast([P, w, P]),
                op=mybir.AluOpType.mult,
            )
            for s in range(w):
                su = cu0 + s
                t = int(tile_of_su1[su])
                k = su - off1[t]
                if k == 0:
                    agg_ps[t] = ppool.tile([P, P], F32, tag="agg1", bufs=3, name=f"agg1_{t}")
                nc.tensor.matmul(
                    out=agg_ps[t][:],
                    lhsT=y[:, s * P : (s + 1) * P],
                    rhs=dg[:, s * P : (s + 1) * P],
                    start=(k == 0),
                    stop=(k == K1[t] - 1),
                )
                if k == K1[t] - 1:
                    l1_epilogue(t)

        # ---- L2 stream (gathers from hm2; waits on all hm2 writes) ----
        agg2_ps = {}
        for cu0, w in call_segments(0, n2sub, CALLW2):
            xh = xpool.tile([P, w * P], BF16, tag="x2")
            gather(xh[:], w, hm2_d, ix2[:, cu0 * 8 : (cu0 + w) * 8])
            dg2 = dpool.tile([P, w * P], BF16, tag="dg2")
            nc.vector.tensor_tensor(
                out=dg2[:],
                in0=ident[:].unsqueeze(1).to_broadcast([P, w, P]),
                in1=s2[:, cu0 : cu0 + w].unsqueeze(2).to_broadcast([P, w, P]),
                op=mybir.AluOpType.mult,
            )
            for s in range(w):
                su = cu0 + s
                t = int(tile_of_su2[su])
                k = su - off2[t]
                if k == 0:
                    agg2_ps[t] = ppool.tile([P, P], F32, tag="agg2", bufs=4, name=f"agg2_{t}")
                nc.tensor.matmul(
                    out=agg2_ps[t][:],
                    lhsT=xh[:, s * P : (s + 1) * P],
                    rhs=dg2[:, s * P : (s + 1) * P],
                    start=(k == 0),
                    stop=(k == K2[t] - 1),
                )

        for t in range(T2):
            a2 = spool.tile([P, P], BF16, tag="a2")
            nc.vector.tensor_copy(out=a2[:], in_=agg2_ps[t][:])
            op_ = ppool.tile([P, P], F32, tag="wout", bufs=1)
            nc.tensor.matmul(out=op_[:], lhsT=a2[:], rhs=W1_s[:], start=True, stop=True)
            ot = spool.tile([P, P], F32, tag="ot")
            nc.scalar.activation(
                out=ot[:], in_=op_[:],
                func=mybir.ActivationFunctionType.Copy, scale=r2m[:, t : t + 1],
            )
            h0f = spool.tile([P, P], F32, tag="h0f")
            nc.vector.tensor_copy(out=h0f[:], in_=h0t[:, t * P : (t + 1) * P])
            nc.vector.tensor_tensor(out=ot[:], in0=ot[:], in1=h0f[:], op=mybir.AluOpType.add)
            if has_b1:
                nc.vector.tensor_tensor(out=ot[:], in0=ot[:], in1=b1bc[:], op=mybir.AluOpType.add)
            nc.sync.dma_start(out=out_d[t * P : (t + 1) * P, :], in_=ot[:])

    # Assign each dma_gather's SWDGE queue from its Tile-assigned DMASW sem
    # lane (lane k -> queue k % NQ) so every sem lane sees exactly one queue.
    from concourse.tile_sem_assignment import PROC_NAME_TO_IDX

    dmasw_base = PROC_NAME_TO_IDX["DMASW0"]
    for inst in nc.inst_map.values():
        if isinstance(inst, mybir.InstDMAGatherAnt):
            proc = getattr(inst, "bass_scheduled_proc", None)
            if proc is not None and dmasw_base <= proc < dmasw_base + 8:
                inst.queue_num = (proc - dmasw_base) % NQ

    nc.compile()
    return nc


def make_in_maps(inputs, plan, has_b0, has_b1):
    emb = np.asarray(inputs["emb"], np.float32)
    W0 = np.asarray(inputs["W0"], np.float32).astype(BF16_NP)
    W1 = np.asarray(inputs["W1"], np.float32).astype(BF16_NP)
    ident = np.eye(P, dtype=np.float32).astype(BF16_NP)
    nrowsA, nrowsB = plan["nrowsA"], plan["nrowsB"]
    in_maps = []
    for md in plan["cores"]:
        tabA = np.zeros((nrowsA, D), BF16_NP)
        tabA[: len(md["rowsA"])] = emb[md["rowsA"]].astype(BF16_NP)
        tabB = np.zeros((nrowsB, D), BF16_NP)
        tabB[: len(md["rowsB"])] = emb[md["rowsB"]].astype(BF16_NP)

        # landing order = global column-major [su][p]; call segmentation is
        # column-contiguous so per-call slices of the wrapped array line up.
        grid = np.concatenate([md["idx1A"], md["idx1B"]], axis=1)  # [P, n1sub]
        ix1 = wrap_idxs(grid.T.ravel())
        ixm = wrap_idxs(np.concatenate([md["midA"], md["midB"]]))
        ix2 = wrap_idxs(md["j2"].T.ravel())
        ixh = wrap_idxs(md["h0idx"])

        m = dict(
            tabA=tabA, tabB=tabB, W0=W0, W1=W1, ident=ident,
            s1meta=md["s1meta"].astype(BF16_NP),
            s2meta=md["s2meta"].astype(BF16_NP),
            ix1=ix1, ixm=ixm, ix2=ix2, ixh=ixh,
        )
        if has_b0:
            m["b0bc"] = np.broadcast_to(np.asarray(inputs["b0"], np.float32), (P, D)).copy()
        if has_b1:
            m["b1bc"] = np.broadcast_to(2.0 * np.asarray(inputs["b1"], np.float32), (P, D)).copy()
        in_maps.append(m)
    return in_maps


def assemble_output(plan, core_outs):
    B = 4096
    out = np.zeros((B, D), np.float32)
    for c, md in enumerate(plan["cores"]):
        co = core_outs[c]
        for t in range(plan["T2"]):
            out[md["orig_seeds"][t]] = co[t * P : (t + 1) * P]
    return out


_CACHE = {}


def _plan_key(inputs):
    h = hashlib.sha1()
    for k in ("nid_src1", "nid_src2", "nid_dst2", "e1_src", "e1_dst", "e2_src", "e2_dst", "b0", "b1"):
        a = np.ascontiguousarray(np.asarray(inputs[k]))
        h.update(k.encode())
        h.update(str(a.shape).encode())
        h.update(a.tobytes())
    return h.hexdigest()


def _get_compiled(inputs):
    key = _plan_key(inputs)
    if key not in _CACHE:
        pl = build_plan(inputs)
        has_b0 = bool(np.any(np.asarray(inputs["b0"]) != 0))
        has_b1 = bool(np.any(np.asarray(inputs["b1"]) != 0))
        nc = build_nc(pl, has_b0, has_b1)
        _CACHE[key] = (pl, has_b0, has_b1, nc)
    return _CACHE[key]


def run_kernel(inputs, trace=False, tmpdir=None):
    pl, has_b0, has_b1, nc = _get_compiled(inputs)
    in_maps = make_in_maps(inputs, pl, has_b0, has_b1)
    res = run_bass_kernel_spmd(
        nc, in_maps, core_ids=list(range(NCORES)), trace=trace, tmpdir=tmpdir
    )
    core_outs = [res.results[c]["out"] for c in range(NCORES)]
    out = assemble_output(pl, core_outs)
    return out, res


def kernel(**inputs):
    out, _ = run_kernel(inputs, trace=False)
    return out
